# revision 1
# baseline (speedup 1.0000x reference)
"""BFFN (linear-attention style gated FFN) Trainium2 Bass kernel, 8 NeuronCores.

Reference computation (all fp32, B=4, N=4096, D=E=1024):
    query = (x_real @ Wqr) * (x_imag @ Wqi)        # [b, n, e]
    key   = x_real @ Wk                             # [b, n, d]
    value = x_imag @ Wv                             # [b, n, e]
    kv    = einsum('bnd,bne->bde', key, value)      # [b, d, e]
    out   = einsum('bnd,bde->bne', query, kv)       # [b, n, e]

Sharding: 8 cores = 4 batches x 2 sequence-halves (n in [0,2048) / [2048,4096)).
Each core computes its (b, half) chunk end-to-end; the per-batch kv reduction
needs the full sequence, so the two cores of a pair AllReduce their partial
kv [d, e] (bf16, 2MB) while the query matmuls run.

Per-core dataflow (all matmuls bf16 operands, fp32 PSUM accumulation):
  x chunks are DMA-cast f32->bf16 on load, then xbar-DMA-transposed to
  xT [d, n] (d on partitions) which serves as:
    - lhsT for key/value matmuls     (out [n, e] natural)
    - rhs  for the queryT matmuls    (out [e_q, n] transposed, lhsT = Wq)
  kv     = sum_nt key[nt].T @ value[nt]   (key natural is already lhsT layout)
  out    = sum_et queryT[et].T @ kv[et]   (kv natural is already rhs layout)
"""
import numpy as np

import concourse.bass as bass
import concourse.mybir as mybir
import concourse.tile as tile
from concourse import bacc
from concourse.bass import ts, ds
from concourse.bass_utils import run_bass_kernel_spmd

F32 = mybir.dt.float32
BF16 = mybir.dt.bfloat16

B, N, D, E = 4, 4096, 1024, 1024
N_CORES = 8
NL = N // 2          # 2048 rows (sequence) per core
P = 128
NT = NL // P         # 16 n-tiles
DT = D // P          # 8 d_in tiles
ET = E // P          # 8 e (and e_q) tiles
FD = 512             # matmul moving free dim / PSUM bank
NCH = NL // FD       # 4 n-chunks of 512

REPLICA_GROUPS = [[0, 1], [2, 3], [4, 5], [6, 7]]


def build_bass():
    nc = bacc.Bacc("TRN2", target_bir_lowering=False, debug=False,
                   num_devices=N_CORES)

    xr = nc.dram_tensor("xr", [NL, D], F32, kind="ExternalInput").ap()
    xi = nc.dram_tensor("xi", [NL, D], F32, kind="ExternalInput").ap()
    wqr = nc.dram_tensor("wqr", [D, E], F32, kind="ExternalInput").ap()
    wqi = nc.dram_tensor("wqi", [D, E], F32, kind="ExternalInput").ap()
    wk = nc.dram_tensor("wk", [D, E], F32, kind="ExternalInput").ap()
    wv = nc.dram_tensor("wv", [D, E], F32, kind="ExternalInput").ap()
    out = nc.dram_tensor("out", [NL, E], F32, kind="ExternalOutput").ap()

    def as_tiles(w):  # [1024, n] DRAM view -> [128, 8, n] partition-major
        return w.rearrange("(t p) n -> p t n", p=P)

    with tile.TileContext(nc) as tc:
        with (
            tc.tile_pool(name="xnat", bufs=3) as xnat_pool,
            tc.tile_pool(name="xt", bufs=2) as xt_pool,
            tc.tile_pool(name="wp", bufs=2) as w_pool,
            tc.tile_pool(name="kvin", bufs=2) as kvin_pool,
            tc.tile_pool(name="kvp", bufs=1) as kv_pool,
            tc.tile_pool(name="prst", bufs=3) as prt_pool,
            tc.tile_pool(name="outst", bufs=3) as out_pool,
            tc.tile_pool(name="pmm", bufs=4, space="PSUM") as pmm,
            tc.tile_pool(name="dram", bufs=2, space="DRAM") as dram_pool,
        ):
            # ---- weight loads (DMA casts f32->bf16 in flight) ----
            wk_sb = w_pool.tile([P, DT, E], BF16, tag="w", name="wk_sb")
            nc.gpsimd.dma_start(wk_sb[:], as_tiles(wk))
            wv_sb = w_pool.tile([P, DT, E], BF16, tag="w", name="wv_sb")
            nc.gpsimd.dma_start(wv_sb[:], as_tiles(wv))

            # ---- load x chunks (cast) + xbar-transpose to xT [d, n] ----
            xtr = xt_pool.tile([P, DT, NL], BF16, tag="xt", name="xtr")
            xti = xt_pool.tile([P, DT, NL], BF16, tag="xt", name="xti")
            for nt in range(NT):
                for src, xt_sb in ((xr, xtr), (xi, xti)):
                    x_nat = xnat_pool.tile([P, D], BF16, tag="xn", name="x_nat")
                    nc.gpsimd.dma_start(x_nat[:], src[ts(nt, P), :])
                    for d in range(DT):
                        nc.sync.dma_start(
                            xt_sb[:, d, ts(nt, P)], x_nat[:, ts(d, P)],
                            transpose=True,
                        )

            # ---- key / value matmuls: out [n, e] natural ----
            key_sb = kvin_pool.tile([P, NT, E], BF16, tag="kvin", name="key_sb")
            val_sb = kvin_pool.tile([P, NT, E], BF16, tag="kvin", name="val_sb")
            for xt_sb, w_sb, o_sb in ((xtr, wk_sb, key_sb), (xti, wv_sb, val_sb)):
                for nt in range(NT):
                    ps = [pmm.tile([P, FD], F32, tag="ps", name="ps_kv")
                          for _ in range(2)]
                    for d in range(DT):
                        lhsT = xt_sb[:, d, ts(nt, P)]
                        for eh in range(2):
                            nc.tensor.matmul(
                                ps[eh][:], lhsT, w_sb[:, d, ts(eh, FD)],
                                start=(d == 0), stop=(d == DT - 1),
                            )
                    for eh in range(2):
                        nc.vector.tensor_copy(o_sb[:, nt, ts(eh, FD)], ps[eh][:])

            # ---- kv partial: kv[d, e] = sum_nt key[nt].T @ value[nt] ----
            kv_part = kv_pool.tile([P, DT, E], BF16, tag="kv", name="kv_part")
            for dt in range(DT):
                ps = [pmm.tile([P, FD], F32, tag="ps", name="ps_kvr")
                      for _ in range(2)]
                for nt in range(NT):
                    lhsT = key_sb[:, nt, ts(dt, P)]
                    for eh in range(2):
                        nc.tensor.matmul(
                            ps[eh][:], lhsT, val_sb[:, nt, ts(eh, FD)],
                            start=(nt == 0), stop=(nt == NT - 1),
                        )
                for eh in range(2):
                    nc.vector.tensor_copy(kv_part[:, dt, ts(eh, FD)], ps[eh][:])

            # ---- pairwise AllReduce of kv (bf16, 2MB) ----
            bounce_in = dram_pool.tile([D, E], BF16, name="bounce_in")
            bounce_out = dram_pool.tile([D, E], BF16, name="bounce_out")
            nc.sync.dma_start(as_tiles(bounce_in), kv_part[:])
            nc.gpsimd.collective_compute(
                "AllReduce",
                mybir.AluOpType.add,
                replica_groups=REPLICA_GROUPS,
                ins=[bounce_in.opt()],
                outs=[bounce_out.opt()],
            )

            # ---- queryT (overlaps the collective): [e_q, n] transposed ----
            wqr_sb = w_pool.tile([P, DT, E], BF16, tag="w", name="wqr_sb")
            nc.gpsimd.dma_start(wqr_sb[:], as_tiles(wqr))
            wqi_sb = w_pool.tile([P, DT, E], BF16, tag="w", name="wqi_sb")
            nc.gpsimd.dma_start(wqi_sb[:], as_tiles(wqi))

            qt_sb = kvin_pool.tile([P, ET, NL], BF16, tag="kvin", name="qt_sb")
            for et in range(ET):
                for nch in range(NCH):
                    ps_r = pmm.tile([P, FD], F32, tag="ps", name="ps_qr")
                    for d in range(DT):
                        nc.tensor.matmul(
                            ps_r[:], wqr_sb[:, d, ts(et, P)],
                            xtr[:, d, ts(nch, FD)],
                            start=(d == 0), stop=(d == DT - 1),
                        )
                    prt = prt_pool.tile([P, FD], BF16, tag="prt", name="prt")
                    nc.vector.tensor_copy(prt[:], ps_r[:])
                    ps_i = pmm.tile([P, FD], F32, tag="ps", name="ps_qi")
                    for d in range(DT):
                        nc.tensor.matmul(
                            ps_i[:], wqi_sb[:, d, ts(et, P)],
                            xti[:, d, ts(nch, FD)],
                            start=(d == 0), stop=(d == DT - 1),
                        )
                    nc.vector.tensor_mul(
                        out=qt_sb[:, et, ts(nch, FD)], in0=prt[:], in1=ps_i[:],
                    )

            # ---- gather reduced kv, final matmul out = queryT.T @ kv ----
            kv_full = kv_pool.tile([P, DT, E], BF16, tag="kv", name="kv_full")
            nc.sync.dma_start(kv_full[:], as_tiles(bounce_out))

            for nt in range(NT):
                ps = [pmm.tile([P, FD], F32, tag="ps", name="ps_out")
                      for _ in range(2)]
                for et in range(ET):
                    lhsT = qt_sb[:, et, ts(nt, P)]
                    for eh in range(2):
                        nc.tensor.matmul(
                            ps[eh][:], lhsT, kv_full[:, et, ts(eh, FD)],
                            start=(et == 0), stop=(et == ET - 1),
                        )
                for eh in range(2):
                    o_st = out_pool.tile([P, FD], F32, tag="ost", name="o_st")
                    nc.vector.tensor_copy(o_st[:], ps[eh][:])
                    nc.sync.dma_start(out[ts(nt, P), ts(eh, FD)], o_st[:])

    nc.compile()
    return nc


def make_in_maps(x_real, x_imag, w_query_real, w_query_imag, w_key, w_value):
    ws = {
        "wqr": np.ascontiguousarray(w_query_real, dtype=np.float32),
        "wqi": np.ascontiguousarray(w_query_imag, dtype=np.float32),
        "wk": np.ascontiguousarray(w_key, dtype=np.float32),
        "wv": np.ascontiguousarray(w_value, dtype=np.float32),
    }
    in_maps = []
    for c in range(N_CORES):
        b, h = divmod(c, 2)
        sl = slice(h * NL, (h + 1) * NL)
        in_maps.append({
            "xr": np.ascontiguousarray(x_real[b, sl], dtype=np.float32),
            "xi": np.ascontiguousarray(x_imag[b, sl], dtype=np.float32),
            **ws,
        })
    return in_maps


def gather_out(results):
    out = np.empty((B, N, E), np.float32)
    for c in range(N_CORES):
        b, h = divmod(c, 2)
        out[b, h * NL:(h + 1) * NL] = results[c]["out"]
    return out


def kernel(x_real, x_imag, w_query_real, w_query_imag, w_key, w_value):
    nc = build_bass()
    in_maps = make_in_maps(x_real, x_imag, w_query_real, w_query_imag,
                           w_key, w_value)
    res = run_bass_kernel_spmd(nc, in_maps, core_ids=list(range(N_CORES)))
    return gather_out(res.results)


if __name__ == "__main__":
    rng = np.random.default_rng(0)
    args = dict(
        x_real=rng.standard_normal((B, N, D), dtype=np.float32),
        x_imag=rng.standard_normal((B, N, D), dtype=np.float32),
        w_query_real=(rng.standard_normal((D, E), dtype=np.float32) / D),
        w_query_imag=(rng.standard_normal((D, E), dtype=np.float32) / D),
        w_key=(rng.standard_normal((D, E), dtype=np.float32) / D),
        w_value=(rng.standard_normal((D, E), dtype=np.float32) / D),
    )
    got = kernel(**args)
    q = np.einsum("bnd,de->bne", args["x_real"], args["w_query_real"]) * \
        np.einsum("bnd,de->bne", args["x_imag"], args["w_query_imag"])
    k = np.einsum("bnd,de->bne", args["x_real"], args["w_key"])
    v = np.einsum("bnd,de->bne", args["x_imag"], args["w_value"])
    kv = np.einsum("bnd,bne->bde", k, v)
    want = np.einsum("bnd,bde->bne", q, kv)
    denom = np.abs(want).max()
    print("max abs err:", np.abs(got - want).max())
    print("rel err:", np.abs(got - want).max() / denom)


# revision 3
# speedup vs baseline: 1.4106x; 1.4106x over previous
"""BFFN (linear-attention style gated FFN) Trainium2 Bass kernel, 8 NeuronCores.

Reference computation (all fp32, B=4, N=4096, D=E=1024):
    query = (x_real @ Wqr) * (x_imag @ Wqi)        # [b, n, e]
    key   = x_real @ Wk                             # [b, n, d]
    value = x_imag @ Wv                             # [b, n, e]
    kv    = einsum('bnd,bne->bde', key, value)      # [b, d, e]
    out   = einsum('bnd,bde->bne', query, kv)       # [b, n, e]

Key algebraic restructure: kv = Wk^T @ (xr^T @ xi) @ Wv.  With
S = xr^T @ xi (the only sequence-length reduction), the kv path costs
N*D*D + 2*D*D*E instead of 2*N*D*E + N*D*E FLOPs, and S is computed from
x in NATURAL layout (lhsT = xr tile, rhs = xi tile — no transposes).

Sharding: 8 cores = 4 batches x 2 sequence-halves. Each pair AllReduces its
partial S (bf16, 2MB) while the query matmuls run; both cores then compute
kv = Wk^T S Wv redundantly (small) and their own half of the output.

Orientations (out = lhsT.T @ rhs, lhsT/rhs contraction dim on partitions):
    S[d,d']    : lhsT = xr_nat[n,d] slice, rhs = xi_nat[n,d'] slice
    queryT[e,n]: lhsT = Wq[d,e] slice,     rhs = xT[d,n] slice (xbar transp.)
    UT[d',dk]  : lhsT = S[d,d'] slice,     rhs = Wk[d,dk] slice
    kv[dk,e]   : lhsT = UT[d',dk] slice,   rhs = Wv[d',e] slice
    out[n,e]   : lhsT = queryT[e,n] slice, rhs = kv[e,...] slice
"""
import numpy as np

import concourse.bass as bass
import concourse.mybir as mybir
import concourse.tile as tile
from concourse import bacc
from concourse.bass import ts, ds
from concourse.bass_utils import run_bass_kernel_spmd

F32 = mybir.dt.float32
BF16 = mybir.dt.bfloat16

B, N, D, E = 4, 4096, 1024, 1024
N_CORES = 8
NL = N // 2          # 2048 rows (sequence) per core
P = 128
NT = NL // P         # 16 n-tiles
DT = D // P          # 8 d tiles
ET = E // P          # 8 e tiles
FD = 512             # matmul moving free dim / PSUM bank
NCH = NL // FD       # 4 n-chunks of 512

REPLICA_GROUPS = [[0, 1], [2, 3], [4, 5], [6, 7]]


def build_bass():
    nc = bacc.Bacc("TRN2", target_bir_lowering=False, debug=False,
                   num_devices=N_CORES)

    xr = nc.dram_tensor("xr", [NL, D], F32, kind="ExternalInput").ap()
    xi = nc.dram_tensor("xi", [NL, D], F32, kind="ExternalInput").ap()
    wqr = nc.dram_tensor("wqr", [D, E], F32, kind="ExternalInput").ap()
    wqi = nc.dram_tensor("wqi", [D, E], F32, kind="ExternalInput").ap()
    wk = nc.dram_tensor("wk", [D, E], F32, kind="ExternalInput").ap()
    wv = nc.dram_tensor("wv", [D, E], F32, kind="ExternalInput").ap()
    out = nc.dram_tensor("out", [NL, E], F32, kind="ExternalOutput").ap()

    def as_tiles(w):  # [1024, n] DRAM view -> [128, 8, n] partition-major
        return w.rearrange("(t p) n -> p t n", p=P)

    with tile.TileContext(nc) as tc:
        with (
            tc.tile_pool(name="xnat", bufs=4) as xnat_pool,
            tc.tile_pool(name="xnh", bufs=3) as xnh_pool,
            tc.tile_pool(name="xt", bufs=2) as xt_pool,
            tc.tile_pool(name="wp", bufs=2) as w_pool,
            tc.tile_pool(name="qt", bufs=1) as qt_pool,
            tc.tile_pool(name="sm", bufs=2) as sm_pool,
            tc.tile_pool(name="sst", bufs=3) as sst_pool,
            tc.tile_pool(name="prst", bufs=3) as prt_pool,
            tc.tile_pool(name="outst", bufs=3) as out_pool,
            tc.tile_pool(name="ps", bufs=8, space="PSUM") as ps_pool,
            tc.tile_pool(name="dram", bufs=2, space="DRAM") as dram_pool,
        ):
            bounce_in = dram_pool.tile([D, D], BF16, name="bounce_in")
            bounce_out = dram_pool.tile([D, D], BF16, name="bounce_out")

            xtr = xt_pool.tile([P, DT, NL], BF16, tag="xt", name="xtr")
            xti = xt_pool.tile([P, DT, NL], BF16, tag="xt", name="xti")

            # ---- pass A over n: load+cast x, xbar-transpose, S[:, 0:512] ----
            ps_s = [ps_pool.tile([P, FD], F32, tag="ps", name="ps_s")
                    for _ in range(DT)]
            for nt in range(NT):
                xr_nat = xnat_pool.tile([P, D], BF16, tag="xn", name="xr_nat")
                nc.gpsimd.dma_start(xr_nat[:], xr[ts(nt, P), :])
                xi_nat = xnat_pool.tile([P, D], BF16, tag="xn", name="xi_nat")
                nc.gpsimd.dma_start(xi_nat[:], xi[ts(nt, P), :])
                nc.sync.dma_start(xtr[:, :, ts(nt, P)], xr_nat[:],
                                  transpose=True)
                nc.sync.dma_start(xti[:, :, ts(nt, P)], xi_nat[:],
                                  transpose=True)
                for d in range(DT):
                    nc.tensor.matmul(
                        ps_s[d][:], xr_nat[:, ts(d, P)], xi_nat[:, :FD],
                        start=(nt == 0), stop=(nt == NT - 1),
                    )
                if nt == 2:
                    # prefetch query weights while x streams
                    wqr_sb = w_pool.tile([P, DT, E], BF16, tag="w",
                                         name="wqr_sb")
                    nc.gpsimd.dma_start(wqr_sb[:], as_tiles(wqr))
                    wqi_sb = w_pool.tile([P, DT, E], BF16, tag="w",
                                         name="wqi_sb")
                    nc.gpsimd.dma_start(wqi_sb[:], as_tiles(wqi))
            for d in range(DT):
                s_st = sst_pool.tile([P, FD], BF16, tag="sst", name="s_st")
                nc.vector.tensor_copy(s_st[:], ps_s[d][:])
                nc.sync.dma_start(bounce_in[ts(d, P), :FD], s_st[:])

            # ---- pass B over n: reload x, S[:, 512:1024] ----
            ps_s2 = [ps_pool.tile([P, FD], F32, tag="ps", name="ps_s2")
                     for _ in range(DT)]
            for nt in range(NT):
                xr_nat2 = xnat_pool.tile([P, D], BF16, tag="xn", name="xr_nat2")
                nc.gpsimd.dma_start(xr_nat2[:], xr[ts(nt, P), :])
                xi_h = xnh_pool.tile([P, FD], BF16, tag="xnh", name="xi_h")
                nc.gpsimd.dma_start(xi_h[:], xi[ts(nt, P), FD:])
                for d in range(DT):
                    nc.tensor.matmul(
                        ps_s2[d][:], xr_nat2[:, ts(d, P)], xi_h[:],
                        start=(nt == 0), stop=(nt == NT - 1),
                    )
            for d in range(DT):
                s_st2 = sst_pool.tile([P, FD], BF16, tag="sst", name="s_st2")
                nc.vector.tensor_copy(s_st2[:], ps_s2[d][:])
                nc.sync.dma_start(bounce_in[ts(d, P), FD:], s_st2[:])

            # ---- pairwise AllReduce of S (bf16, 2MB) ----
            nc.gpsimd.collective_compute(
                "AllReduce",
                mybir.AluOpType.add,
                replica_groups=REPLICA_GROUPS,
                ins=[bounce_in.opt()],
                outs=[bounce_out.opt()],
            )

            # ---- queryT (overlaps the collective): [e_q, n] ----
            qt_sb = qt_pool.tile([P, ET, NL], BF16, tag="qt", name="qt_sb")
            for et in range(ET):
                for nch in range(NCH):
                    ps_r = ps_pool.tile([P, FD], F32, tag="ps", name="ps_qr")
                    for d in range(DT):
                        nc.tensor.matmul(
                            ps_r[:], wqr_sb[:, d, ts(et, P)],
                            xtr[:, d, ts(nch, FD)],
                            start=(d == 0), stop=(d == DT - 1),
                        )
                    prt = prt_pool.tile([P, FD], BF16, tag="prt", name="prt")
                    nc.vector.tensor_copy(prt[:], ps_r[:])
                    ps_i = ps_pool.tile([P, FD], F32, tag="ps", name="ps_qi")
                    for d in range(DT):
                        nc.tensor.matmul(
                            ps_i[:], wqi_sb[:, d, ts(et, P)],
                            xti[:, d, ts(nch, FD)],
                            start=(d == 0), stop=(d == DT - 1),
                        )
                    nc.vector.tensor_mul(
                        out=qt_sb[:, et, ts(nch, FD)], in0=prt[:], in1=ps_i[:],
                    )

            # ---- kv = Wk^T S Wv from the reduced S ----
            wk_sb = w_pool.tile([P, DT, E], BF16, tag="w", name="wk_sb")
            nc.gpsimd.dma_start(wk_sb[:], as_tiles(wk))
            wv_sb = w_pool.tile([P, DT, E], BF16, tag="w", name="wv_sb")
            nc.gpsimd.dma_start(wv_sb[:], as_tiles(wv))

            s_sb = sm_pool.tile([P, DT, D], BF16, tag="sm", name="s_sb")
            nc.sync.dma_start(s_sb[:], as_tiles(bounce_out))

            ut_sb = sm_pool.tile([P, DT, D], BF16, tag="sm", name="ut_sb")
            for dpt in range(DT):      # d' tile (UT partition dim)
                ps_u = [ps_pool.tile([P, FD], F32, tag="ps", name="ps_u")
                        for _ in range(2)]
                for d in range(DT):
                    lhsT = s_sb[:, d, ts(dpt, P)]
                    for kh in range(2):
                        nc.tensor.matmul(
                            ps_u[kh][:], lhsT, wk_sb[:, d, ts(kh, FD)],
                            start=(d == 0), stop=(d == DT - 1),
                        )
                for kh in range(2):
                    nc.vector.tensor_copy(ut_sb[:, dpt, ts(kh, FD)],
                                          ps_u[kh][:])

            kv_sb = sm_pool.tile([P, DT, E], BF16, tag="sm", name="kv_sb")
            for dkt in range(DT):      # dk tile (kv partition dim)
                ps_k = [ps_pool.tile([P, FD], F32, tag="ps", name="ps_k")
                        for _ in range(2)]
                for dp in range(DT):
                    lhsT = ut_sb[:, dp, ts(dkt, P)]
                    for eh in range(2):
                        nc.tensor.matmul(
                            ps_k[eh][:], lhsT, wv_sb[:, dp, ts(eh, FD)],
                            start=(dp == 0), stop=(dp == DT - 1),
                        )
                for eh in range(2):
                    nc.vector.tensor_copy(kv_sb[:, dkt, ts(eh, FD)],
                                          ps_k[eh][:])

            # ---- out = queryT.T @ kv ----
            for nt in range(NT):
                ps_o = [ps_pool.tile([P, FD], F32, tag="ps", name="ps_o")
                        for _ in range(2)]
                for et in range(ET):
                    lhsT = qt_sb[:, et, ts(nt, P)]
                    for eh in range(2):
                        nc.tensor.matmul(
                            ps_o[eh][:], lhsT, kv_sb[:, et, ts(eh, FD)],
                            start=(et == 0), stop=(et == ET - 1),
                        )
                for eh in range(2):
                    o_st = out_pool.tile([P, FD], F32, tag="ost", name="o_st")
                    nc.vector.tensor_copy(o_st[:], ps_o[eh][:])
                    nc.sync.dma_start(out[ts(nt, P), ts(eh, FD)], o_st[:])

    nc.compile()
    return nc


def make_in_maps(x_real, x_imag, w_query_real, w_query_imag, w_key, w_value):
    ws = {
        "wqr": np.ascontiguousarray(w_query_real, dtype=np.float32),
        "wqi": np.ascontiguousarray(w_query_imag, dtype=np.float32),
        "wk": np.ascontiguousarray(w_key, dtype=np.float32),
        "wv": np.ascontiguousarray(w_value, dtype=np.float32),
    }
    in_maps = []
    for c in range(N_CORES):
        b, h = divmod(c, 2)
        sl = slice(h * NL, (h + 1) * NL)
        in_maps.append({
            "xr": np.ascontiguousarray(x_real[b, sl], dtype=np.float32),
            "xi": np.ascontiguousarray(x_imag[b, sl], dtype=np.float32),
            **ws,
        })
    return in_maps


def gather_out(results):
    out = np.empty((B, N, E), np.float32)
    for c in range(N_CORES):
        b, h = divmod(c, 2)
        out[b, h * NL:(h + 1) * NL] = results[c]["out"]
    return out


def kernel(x_real, x_imag, w_query_real, w_query_imag, w_key, w_value):
    nc = build_bass()
    in_maps = make_in_maps(x_real, x_imag, w_query_real, w_query_imag,
                           w_key, w_value)
    res = run_bass_kernel_spmd(nc, in_maps, core_ids=list(range(N_CORES)))
    return gather_out(res.results)


if __name__ == "__main__":
    rng = np.random.default_rng(0)
    args = dict(
        x_real=rng.standard_normal((B, N, D), dtype=np.float32),
        x_imag=rng.standard_normal((B, N, D), dtype=np.float32),
        w_query_real=(rng.standard_normal((D, E), dtype=np.float32) / D),
        w_query_imag=(rng.standard_normal((D, E), dtype=np.float32) / D),
        w_key=(rng.standard_normal((D, E), dtype=np.float32) / D),
        w_value=(rng.standard_normal((D, E), dtype=np.float32) / D),
    )
    got = kernel(**args)
    q = np.einsum("bnd,de->bne", args["x_real"], args["w_query_real"]) * \
        np.einsum("bnd,de->bne", args["x_imag"], args["w_query_imag"])
    k = np.einsum("bnd,de->bne", args["x_real"], args["w_key"])
    v = np.einsum("bnd,de->bne", args["x_imag"], args["w_value"])
    kv = np.einsum("bnd,bne->bde", k, v)
    want = np.einsum("bnd,bde->bne", q, kv)
    denom = np.abs(want).max()
    print("max abs err:", np.abs(got - want).max())
    print("rel err:", np.abs(got - want).max() / denom)
